# revision 35
# baseline (speedup 1.0000x reference)
"""Trainium2 Bass kernel for nn_DecoderOutLayer (per-frequency causal 5-tap
temporal conv: x[B,C,T,F], weight[F,C*5,O], b[F,O] -> out[B,O,T,F]).

Sharding: frequency axis F=96 split across 8 cores (12 freqs/core).

Per-core algorithm (fp16 split-precision, fp32 PSUM accumulate):
  Host packs x as [fp=6, (f2,c)=128, b=4, 4+t] (f-pair x channel on
  partitions, 4-zero causal pad baked in), split into fp16 hi + lo parts
  (x = x_hi + x_lo exactly to ~2^-24), same for the weights.  Device:
   pass 1: for each (fp, b, t-window of 128): three accumulating matmuls
     x_hi@W_hi + x_hi@W_lo + x_lo@W_hi (the dropped lo*lo term is ~2^-24),
     with lhsT = X window (stationary, [128 part=(f2,c), 128 cols=t]) and
     rhs = W_exp[fp] [128, 20], cols = (s=5 shifts, f2, o), where
     W_exp[(f2,c),(s,f2',o)] = delta(f2,f2') * w[f, c, 4-s, o].
     psum1[t_local, (s,f2,o)] then holds the contribution of input time t
     to output time t + s.
   pass 2: shift-matrix matmuls S_s[t, t'] = delta(t = t' + 4 - s)
     contract over the 128 t-partitions to realign the 5 shifted
     contributions onto common output times (again in fp16 hi + lo
     pieces), accumulating in PSUM, plus rank-1 matmuls adding the bias.
     Windows advance by 124 so every output column of a window is
     complete (no cross-window terms).
"""
import os
import sys

for _p in (
    "/root/.axon_site",
    "/root/.axon_site/_ro/trn_rl_repo",
    "/root/.axon_site/_ro/pypackages",
    "/opt/trn_rl_repo",
):
    if os.path.isdir(_p) and _p not in sys.path:
        sys.path.append(_p)

import numpy as np

import concourse.bass as bass  # noqa: F401
import concourse.mybir as mybir
import concourse.tile as tile
from concourse import bacc, bass_utils

TC = 5
B, C, T, F = 4, 64, 1000, 96
O = 2
NCORES = 8
FL = F // NCORES       # 12 freqs per core
NFP = FL // 2          # 6 f-pairs
NTB = 9                # t-windows per b: 8 x stride-124 + tail at 876
WIN = 128
STEP = 124
TPAD = T + 4           # 1004 cols per b in SBUF (4 leading zeros)
NQ = 2 * O             # 4 = (f2, o)
PS2N = 2 * NFP * NTB * NQ   # 432 cols per psum2 bank (2 b's)
F32 = mybir.dt.float32
F16 = mybir.dt.float16
XCOLS = 2 * B * TPAD        # 8032 cols per x tile (2 f-pairs)

# 3 = fp16 hi/lo split (rel err ~1e-6); 1 = single fp16 pass (~3e-4, less
# HBM traffic / faster)
TERMS = int(os.environ.get("KERNEL_TERMS", "3"))


def _split16(a):
    hi = a.astype(np.float16)
    lo = (a - hi.astype(np.float32)).astype(np.float16)
    return hi, lo


def _host_prep(x, weight, b):
    """Full inputs -> per-core input maps (numpy only)."""
    x = np.ascontiguousarray(np.asarray(x, dtype=np.float32))
    weight = np.asarray(weight, dtype=np.float32)
    bias = np.asarray(b, dtype=np.float32)
    x_t = np.ascontiguousarray(x.transpose(3, 1, 0, 2))   # [F,C,B,T]
    w4 = weight.reshape(F, C, TC, O)

    # shift matrices, core-independent: [128, 6*124] (partition-major),
    # exact 0/1 values so fp16 storage is lossless
    sm = np.zeros((6, 128, STEP), np.float16)
    for s in range(TC):
        tp = np.arange(STEP)
        sm[s, tp + 4 - s, tp] = 1.0
    sm[5, 0, :] = 1.0       # bias row-broadcast matrix
    sm_h = np.ascontiguousarray(
        sm.transpose(1, 0, 2).reshape(128, 6 * STEP))

    in_maps = []
    for g in range(NCORES):
        f0 = g * FL
        xg = np.zeros((NFP, 2 * C, B, TPAD), np.float32)
        xg[:, :, :, 4:] = x_t[f0:f0 + FL].reshape(NFP, 2 * C, B, T)
        # -> [3, 128, 2*B*TPAD]: two f-pairs per SBUF tile, partition-major
        xg = np.ascontiguousarray(
            xg.reshape(3, 2, 2 * C, B * TPAD).transpose(0, 2, 1, 3)
        ).reshape(3, 2 * C, XCOLS)
        xh, xl = _split16(xg)

        we = np.zeros((NFP, 128, 20), np.float32)
        w5 = w4[f0:f0 + FL].reshape(NFP, 2, C, TC, O)
        w5s = w5[:, :, :, ::-1, :]                        # tap k -> shift s=4-k
        wev = we.reshape(NFP, 2, C, TC, 2, O)             # fp,f2,c,s,f2',o
        for f2 in range(2):
            wev[:, f2, :, :, f2, :] = w5s[:, f2]
        we = np.ascontiguousarray(we.transpose(1, 0, 2).reshape(128, NFP * 20))
        wh, wl = _split16(we)

        brv = np.zeros((128, PS2N), np.float32)
        v = bias[f0:f0 + FL].reshape(NFP, 2, O)
        arr = np.broadcast_to(v[:, None, :, :], (NFP, NTB, 2, O)).reshape(-1)
        brv[0] = np.tile(arr, 2)
        bh, bl = _split16(brv)

        m = {
            "xinh": xh,
            "wexp": np.ascontiguousarray(np.concatenate([wh, wl], axis=1)),
            "consts": np.ascontiguousarray(
                np.concatenate([sm_h, bh, bl], axis=1)),
        }
        if TERMS == 3:
            m["xinl"] = xl
        in_maps.append(m)
    return in_maps


def build_program(nc):
    """Declare DRAM tensors + emit the Tile program."""
    xinh = nc.dram_tensor("xinh", [3, 128, XCOLS], F16,
                          kind="ExternalInput").ap()
    if TERMS == 3:
        xinl = nc.dram_tensor("xinl", [3, 128, XCOLS], F16,
                              kind="ExternalInput").ap()
    wexp = nc.dram_tensor("wexp", [128, 2 * NFP * 20], F16,
                          kind="ExternalInput").ap()
    ncst = 6 * STEP + 2 * PS2N      # 744 + 864 = 1608
    cst = nc.dram_tensor("consts", [128, ncst], F16, kind="ExternalInput").ap()
    # One DRAM output per b: raw [p, tb, q] window dump, p-major so each
    # partition writes one contiguous 864 B run; the host re-indexes
    # t = tb*124 + p (tb<8) / t = 876 + p (tb=8) and drops overlap rows.
    outs = [nc.dram_tensor(f"out{bb}", [STEP, NTB * FL * O], F32,
                           kind="ExternalOutput").ap() for bb in range(B)]

    with tile.TileContext(nc) as tc:
        from contextlib import ExitStack
        with ExitStack() as ctx:
            const = ctx.enter_context(tc.tile_pool(name="const", bufs=1))
            ps1_pool = ctx.enter_context(
                tc.tile_pool(name="ps1", bufs=6, space="PSUM"))
            ps2_pool = ctx.enter_context(
                tc.tile_pool(name="ps2", bufs=2, space="PSUM"))

            cst_sb = const.tile([128, ncst], F16, name="cst_sb")
            w_all = const.tile([128, 2 * NFP * 20], F16, name="w_all")
            tmp_h = const.tile([128, B * NFP * NTB * 20], F16, name="tmp_h")
            tmp_l = const.tile([128, B * NFP * NTB * 20], F16, name="tmp_l")
            stage = const.tile([128, B * NTB * 24], F32, name="stage")
            xth = [const.tile([128, XCOLS], F16, name=f"xh_sb{j}")
                   for j in range(3)]
            if TERMS == 3:
                xtl = [const.tile([128, XCOLS], F16, name=f"xl_sb{j}")
                       for j in range(3)]
            sm_sb = cst_sb[:, 0:6 * STEP]
            brh_sb = cst_sb[:, 6 * STEP:6 * STEP + PS2N]
            brl_sb = cst_sb[:, 6 * STEP + PS2N:ncst]

            nc.sync.dma_start(cst_sb[:], cst)
            nc.sync.dma_start(w_all[:], wexp)
            for j in range(3):
                nc.sync.dma_start(xth[j][:], xinh[j])
                if TERMS == 3:
                    nc.sync.dma_start(xtl[j][:], xinl[j])

            def xwin(tiles, fp, col0):
                return tiles[fp // 2][:, (fp % 2) * B * TPAD + col0:
                                      (fp % 2) * B * TPAD + col0 + WIN]

            # pass 1
            for fp in range(NFP):
                w_hi = w_all[:, fp * 20:(fp + 1) * 20]
                w_lo = w_all[:, (NFP + fp) * 20:(NFP + fp + 1) * 20]
                for bb in range(B):
                    ps1 = ps1_pool.tile([128, NTB * 20], F32, tag="ps1")
                    for tb in range(NTB):
                        c0 = bb * TPAD + (tb * STEP if tb < 8 else TPAD - WIN)
                        dst = ps1[:, tb * 20:(tb + 1) * 20]
                        xh = xwin(xth, fp, c0)
                        if TERMS == 3:
                            nc.tensor.matmul(dst, lhsT=xh, rhs=w_hi,
                                             start=True, stop=False)
                            nc.tensor.matmul(dst, lhsT=xh, rhs=w_lo,
                                             start=False, stop=False)
                            nc.tensor.matmul(dst, lhsT=xwin(xtl, fp, c0),
                                             rhs=w_hi, start=False, stop=True)
                        else:
                            nc.tensor.matmul(dst, lhsT=xh, rhs=w_hi,
                                             start=True, stop=True)
                    # fp16 hi/lo of psum1 for pass 2; single engine (DVE)
                    # keeps the consumers at one wait semaphore
                    sl = np.s_[:, bb * 1080 + fp * 180:
                               bb * 1080 + (fp + 1) * 180]
                    nc.vector.tensor_copy(tmp_h[sl], ps1[:])
                    if TERMS == 3:
                        nc.vector.tensor_sub(tmp_l[sl], ps1[:], tmp_h[sl])

            # pass 2: tmp col = b*1080 + (fp*9+tb)*20 + s*4 + (f2*2+o)
            tmp4h = tmp_h[:].rearrange("p (b m u) -> p b m u", b=B, m=NFP * NTB)
            tmp4l = tmp_l[:].rearrange("p (b m u) -> p b m u", b=B, m=NFP * NTB)
            for bank in range(2):
                ps2 = ps2_pool.tile([STEP, PS2N], F32, tag="ps2")
                lbias = sm_sb[:, 5 * STEP:6 * STEP]
                nc.tensor.matmul(ps2[:], lhsT=lbias, rhs=brh_sb,
                                 start=True, stop=False)
                nc.tensor.matmul(ps2[:], lhsT=lbias, rhs=brl_sb,
                                 start=False, stop=False)
                for s in range(TC):
                    ssel = np.s_[:, 2 * bank:2 * bank + 2, :,
                                 s * NQ:(s + 1) * NQ]
                    s_mat = sm_sb[:, s * STEP:(s + 1) * STEP]
                    last = s == TC - 1
                    nc.tensor.matmul(ps2[:], lhsT=s_mat, rhs=tmp4h[ssel],
                                     start=False,
                                     stop=(last and TERMS != 3))
                    if TERMS == 3:
                        nc.tensor.matmul(ps2[:], lhsT=s_mat, rhs=tmp4l[ssel],
                                         start=False, stop=last)
                # psum2 col = bl*216 + fp*36 + tb*4 + q -> stage col tb*24+fp*4+q
                for bl in range(2):
                    bb = 2 * bank + bl
                    src = ps2[:, bl * 216:(bl + 1) * 216].rearrange(
                        "p (f m q) -> p m f q", f=NFP, m=NTB)
                    nc.vector.tensor_copy(
                        stage[:STEP, bb * 216:(bb + 1) * 216].rearrange(
                            "p (m f q) -> p m f q", m=NTB, f=NFP), src)

            for bb in range(B):
                nc.sync.dma_start(outs[bb],
                                  stage[:STEP, bb * 216:(bb + 1) * 216])
    return "out"


_CACHED = {}


def _get_nc():
    if "nc" not in _CACHED:
        # bacc.Bacc (not bass.Bass): its compile() pass legalizes multi-wait
        # instructions onto InstEventSemaphore (1 wait/inst ISA limit)
        nc = bacc.Bacc("TRN2", target_bir_lowering=False, debug=False,
                       num_devices=NCORES)
        build_program(nc)
        nc.compile()
        _CACHED["nc"] = nc
    return _CACHED["nc"]


def _gather(results):
    full = np.empty((B, O, T, F), np.float32)
    co = np.empty((B, T, FL * O), np.float32)
    for g in range(NCORES):
        for bb in range(B):
            arr = results[g][f"out{bb}"].reshape(STEP, NTB, FL * O)
            co[bb, :8 * STEP] = arr[:, :8].transpose(1, 0, 2).reshape(
                8 * STEP, FL * O)
            co[bb, 8 * STEP:] = arr[STEP - (T - 8 * STEP):, 8]
        full[:, :, :, g * FL:(g + 1) * FL] = \
            co.reshape(B, T, FL, O).transpose(0, 3, 1, 2)
    return full


def kernel(x, weight, b, **run_kwargs):
    in_maps = _host_prep(x, weight, b)
    nc = _get_nc()
    res = bass_utils.run_bass_kernel_spmd(
        nc, in_maps, core_ids=list(range(NCORES)), **run_kwargs)
    out = _gather(res.results)
    if run_kwargs:
        return out, res
    return out
